# revision 30
# baseline (speedup 1.0000x reference)
# Distributed GQA attention prefill kernel for one TRN2 chip (8 NeuronCores).
#
# Problem: B=2, S=1024, D=2048, H=32 q-heads, KV=4 kv-heads, HD=64, causal,
# RoPE, f32 I/O. Sharding: core d = (batch g=d//4, kv-head kv=d%4). Each core
# computes q-proj for its 8 q heads, k/v-proj for its kv head, attention, and
# a partial o_proj ([S, 512] @ wo[512kv:512kv+512, :]). Four bf16
# ReduceScatters (one per q-block pair, rows of s-chunks {j, j+4} stacked)
# over each group of 4 cores sum the partials while later pairs compute.
#
# Compute runs in bf16 (f32 PSUM accumulation); f32 matmul on TRN2 is 4
# cycles/row vs 1 for bf16. All DRAM loads are gpsimd casting DMAs
# (f32 -> bf16 on the fly). Attention processes 2 heads per matmul (they
# share the kv head) to halve instruction count and keep the PE pipelined.
import sys

import numpy as np

try:
    import concourse.bass as bass  # noqa: F401
except ImportError:
    for p in ("/opt/trn_rl_repo", "/root/.axon_site/_ro/trn_rl_repo"):
        if p not in sys.path:
            sys.path.append(p)
    import concourse.bass as bass  # noqa: F401

import concourse.bacc as bacc
import concourse.mybir as mybir
import concourse.tile as tile
from concourse import masks
from concourse.bass_utils import run_bass_kernel_spmd

S = 1024
D = 2048
H = 32
KV = 4
HD = 64
NH = 8  # q heads per core
P = 128
SC = S // P  # 8 seq chunks
DC = D // P  # 16 D chunks
N_CORES = 8
GROUPS = [[0, 1, 2, 3], [4, 5, 6, 7]]

F32 = mybir.dt.float32
BF16 = mybir.dt.bfloat16

_NC_CACHE = {}


def _build_graph():
    nc = bacc.Bacc("TRN2", target_bir_lowering=False, debug=False, num_devices=N_CORES)

    xt_p = nc.dram_tensor("xt", [SC, P, DC * P], F32, kind="ExternalInput")
    wq_p = nc.dram_tensor("wq", [P, DC * 512], F32, kind="ExternalInput")
    wkv_p = nc.dram_tensor("wkv", [P, DC * 2 * HD], F32, kind="ExternalInput")
    wo_p = nc.dram_tensor("wo", [P, DC * 512], F32, kind="ExternalInput")
    cs_p = nc.dram_tensor("cs8", [P, SC * 256], F32, kind="ExternalInput")
    sn_p = nc.dram_tensor("sn8", [P, SC * 256], F32, kind="ExternalInput")
    mk_p = nc.dram_tensor("mkb", [P, SC * P], F32, kind="ExternalInput")
    out_p = nc.dram_tensor("out", [S, 512], F32, kind="ExternalOutput")

    with tile.TileContext(nc) as tc:
        with (
            tc.tile_pool(name="const", bufs=1) as constp,
            tc.tile_pool(name="big", bufs=1) as bigp,
            tc.tile_pool(name="work", bufs=1) as workp,
            tc.tile_pool(name="rt", bufs=4) as rtp,
            tc.tile_pool(name="attn", bufs=3) as attnp,
            tc.tile_pool(name="opart", bufs=2) as opartp,
            tc.tile_pool(name="tiny", bufs=2) as tinyp,
            tc.tile_pool(name="psum", bufs=1, space="PSUM") as psump,
            tc.tile_pool(name="dram", bufs=1, space="DRAM") as dramp,
        ):
            # ---- constants (cheap engine work first; DMAs ordered by need) ----
            ident = constp.tile([P, P], BF16, tag="ident")
            masks.make_identity(nc, ident[:])
            ones64 = constp.tile([1, 64], BF16, tag="ones64")
            nc.any.memset(ones64[:], 1.0)

            # Bulk loads: plain f32 DMAs on the two HWDGE rings (the gpsimd
            # casting-DMA path is ~4x slower), staged through SBUF and cast
            # to bf16 on ACT/DVE.
            xT_all = bigp.tile([P, SC * DC * P], BF16, tag="xT_all")
            wkv_all = bigp.tile([P, DC * 2 * HD], BF16, tag="wkv_all")
            wq_all = bigp.tile([P, DC * 512], BF16, tag="wq_all")
            wo_all = bigp.tile([P, DC * 512], BF16, tag="wo_all")

            def stage(eng, cast_eng, dst, src):
                stg = opartp.tile([P, 2048], F32, tag="ldstg", bufs=3)
                eng.dma_start(out=stg[:].rearrange("p (a b) -> p a b", a=4), in_=src)
                if cast_eng == "act":
                    nc.scalar.copy(dst, stg[:])
                else:
                    nc.vector.tensor_copy(dst, stg[:])

            def load_xt(s, eng, cast_eng="act"):
                stage(
                    eng,
                    cast_eng,
                    xT_all[:, s * D : (s + 1) * D],
                    xt_p[s, :, :].rearrange("p (a b) -> p a b", a=4),
                )

            def load_wq(g, eng, cast_eng="dve"):
                stage(
                    eng,
                    cast_eng,
                    wq_all[:, g * 2048 : (g + 1) * 2048],
                    wq_p[:, g * 2048 : (g + 1) * 2048].rearrange(
                        "p (a b) -> p a b", a=4
                    ),
                )

            def load_wo(g, eng, cast_eng="act"):
                stage(
                    eng,
                    cast_eng,
                    wo_all[:, g * 2048 : (g + 1) * 2048],
                    wo_p[:, g * 2048 : (g + 1) * 2048].rearrange(
                        "p (a b) -> p a b", a=4
                    ),
                )

            load_xt(0, nc.sync, cast_eng="dve")
            stage(
                nc.scalar,
                "dve",
                wkv_all[:],
                wkv_p[:, :].rearrange("p (a b) -> p a b", a=4),
            )
            load_wq(0, nc.scalar)
            load_xt(1, nc.sync, cast_eng="dve")
            load_wq(1, nc.scalar)
            load_xt(2, nc.sync)
            load_wq(2, nc.scalar)
            load_xt(3, nc.sync)
            load_wq(3, nc.scalar)
            for s in range(4, SC):
                load_xt(s, nc.sync if s % 2 == 0 else nc.scalar)
            cst = constp.tile([P, SC * 256], F32, tag="cst")  # cos, tiled x8 heads
            nc.sync.dma_start(out=cst[:], in_=cs_p[:, :])
            snt = constp.tile([P, SC * 256], F32, tag="snt")
            nc.scalar.dma_start(out=snt[:], in_=sn_p[:, :])
            mkt = constp.tile([P, SC * P], BF16, tag="mkt")  # binary diag masks^T
            nc.gpsimd.dma_start(out=mkt[:], in_=mk_p[:, :])
            for g in range(4):
                load_wo(g, nc.sync if g % 2 == 0 else nc.scalar)

            # ---- projections + RoPE (natural [s, ch] layout) ----
            qrot = [workp.tile([P, NH * HD], BF16, tag=f"qr{s}", name=f"qr{s}") for s in range(SC)]
            krot = [workp.tile([P, HD], BF16, tag=f"kr{s}", name=f"kr{s}") for s in range(SC)]
            vaug = [workp.tile([P, HD + 1], BF16, tag=f"va{s}", name=f"va{s}") for s in range(SC)]

            def rope(ps_ap, dst, s, nh):
                # ps_ap: PSUM AP [128, nh*64] f32; dst: SBUF bf16 same shape
                pv = ps_ap.rearrange("p (h t c) -> p h t c", h=nh, t=2)
                dv = dst[:].rearrange("p (h t c) -> p h t c", h=nh, t=2)
                cs = cst[:, s * 256 : s * 256 + nh * 32].rearrange(
                    "p (h c) -> p h c", h=nh
                )
                sn = snt[:, s * 256 : s * 256 + nh * 32].rearrange(
                    "p (h c) -> p h c", h=nh
                )
                lo, hi = pv[:, :, 0, :], pv[:, :, 1, :]
                t1 = rtp.tile([P, NH * 32], F32, tag="rt1")
                t2 = rtp.tile([P, NH * 32], F32, tag="rt2")
                t1v = t1[:, : nh * 32].rearrange("p (h c) -> p h c", h=nh)
                t2v = t2[:, : nh * 32].rearrange("p (h c) -> p h c", h=nh)
                nc.any.tensor_mul(t1v, lo, cs)
                nc.any.tensor_mul(t2v, hi, sn)
                nc.any.tensor_sub(dv[:, :, 0, :], t1v, t2v)
                nc.any.tensor_mul(t1v, hi, cs)
                nc.any.tensor_mul(t2v, lo, sn)
                nc.any.tensor_add(dv[:, :, 1, :], t1v, t2v)

            qTall = workp.tile([64, NH * S], BF16, tag="qTall")
            kT = workp.tile([64, S], BF16, tag="kT")
            for s in range(SC):
                pkv = psump.tile([P, 2 * HD], F32, tag="mm512", bufs=2)
                for d in range(DC):
                    nc.tensor.matmul(
                        pkv[:],
                        xT_all[:, s * D + d * P : s * D + (d + 1) * P],
                        wkv_all[:, d * 2 * HD : (d + 1) * 2 * HD],
                        start=(d == 0),
                        stop=(d == DC - 1),
                    )
                rope(pkv[:, 0:HD], krot[s], s, 1)
                nc.vector.tensor_copy(vaug[s][:, 0:HD], pkv[:, HD : 2 * HD])
                nc.any.memset(vaug[s][:, HD : HD + 1], 1.0)
                tpk = psump.tile([64, P], BF16, tag="sc", bufs=4)
                nc.tensor.transpose(tpk[:], krot[s][:], ident[:])
                nc.vector.tensor_copy(kT[:, s * P : (s + 1) * P], tpk[:])

            for s in range(SC):
                pq = psump.tile([P, NH * HD], F32, tag="mm512", bufs=2)
                for d in range(DC):
                    nc.tensor.matmul(
                        pq[:],
                        xT_all[:, s * D + d * P : s * D + (d + 1) * P],
                        wq_all[:, d * 512 : (d + 1) * 512],
                        start=(d == 0),
                        stop=(d == DC - 1),
                    )
                rope(pq[:, :], qrot[s], s, NH)
                for h in range(NH):
                    tpq = psump.tile([64, P], BF16, tag="sc", bufs=4)
                    nc.tensor.transpose(
                        tpq[:], qrot[s][:, h * HD : (h + 1) * HD], ident[:]
                    )
                    nc.vector.tensor_copy(
                        qTall[:, h * S + s * P : h * S + (s + 1) * P], tpq[:]
                    )

            # view: [64, a(4), c(2), h(8), i(128)]; a scores matmul's rhs takes
            # (c, h, i)-ordered columns: [j:h | j:h' | j+4:h | j+4:h']
            qview = qTall[:].rearrange("p (h c a b) -> p a c h b", h=NH, c=2, a=4, b=P)

            def _norm_one(j, oa, qd, qb):
                # oa: PSUM [65, 512] (4 heads x 128 cols); row 64 = denominators
                lg = tinyp.tile([1, 512], F32, tag="lg")
                nc.scalar.activation(
                    lg[:], oa[HD : HD + 1, :], mybir.ActivationFunctionType.Ln
                )
                recb = tinyp.tile([1, 512], BF16, tag="recb")
                nc.scalar.activation(
                    recb[:], lg[:], mybir.ActivationFunctionType.Exp, scale=-1.0
                )
                rb = psump.tile([64, 512], F32, tag="sc", bufs=4)
                nc.tensor.matmul(rb[:], ones64[:], recb[:], start=True, stop=True)
                rb_sb = tinyp.tile([64, 512], F32, tag="rbs")
                nc.vector.tensor_copy(rb_sb[:], rb[:])
                for h4 in range(4):
                    h = 4 * qd + h4
                    nc.vector.tensor_mul(
                        oT[h // 2][
                            64 * (h % 2) : 64 * (h % 2) + 64, qb * P : (qb + 1) * P
                        ],
                        oa[0:HD, h4 * P : (h4 + 1) * P],
                        rb_sb[:, h4 * P : (h4 + 1) * P],
                    )

            def _normalize(j, oa_a, oa_b, qd):
                _norm_one(j, oa_a, qd, j)
                _norm_one(j, oa_b, qd, j + 4)

            # ---- attention (2 heads per matmul) + per-pair o_proj and RS ----
            oT = [workp.tile([P, S], BF16, tag=f"oT{c}", name=f"oT{c}") for c in range(4)]
            agin = [dramp.tile([512, 256], BF16, name=f"agin{j}") for j in range(4)]
            agout = [dramp.tile([D, 256], BF16, name=f"agout{j}") for j in range(4)]

            def emit_oproj(j):
                # o_proj for pair j from the AllGathered full-channel oT
                ag_sb = opartp.tile([P, DC * 256], BF16, tag="agsb")
                nc.sync.dma_start(
                    out=ag_sb[:].rearrange("p (c n) -> p c n", c=DC),
                    in_=agout[j][:, :].rearrange("(c p) n -> p c n", p=P),
                )
                for srow in range(2):
                    po = psump.tile([P, 512], F32, tag="mm512", bufs=2)
                    for c16 in range(DC):
                        nc.tensor.matmul(
                            po[:],
                            ag_sb[:, c16 * 256 + srow * P : c16 * 256 + (srow + 1) * P],
                            wo_all[:, c16 * 512 : (c16 + 1) * 512],
                            start=(c16 == 0),
                            stop=(c16 == DC - 1),
                        )
                    osb = opartp.tile([P, 512], F32, tag="osb")
                    nc.vector.tensor_copy(osb[:], po[:])
                    nc.sync.dma_start(
                        out=out_p[256 * j + srow * P : 256 * j + (srow + 1) * P, :],
                        in_=osb[:],
                    )

            pending_oproj = []
            for j in (3, 2, 1, 0):  # q-block pair (j, j+4), heavy first
                pending = None  # deferred normalization: (oa_a, oa_b, qd)
                for qd in range(2):  # head quad (4qd .. 4qd+3)
                    # phase 1: stream all score matmuls + exp into SBUF
                    ats = []
                    for skc in range(j + 5):
                        both = skc <= j
                        ncols = 1024 if both else 512
                        at_t = attnp.tile([P, ncols], BF16, tag="at", bufs=8)
                        if both:
                            sc_a = psump.tile([P, 512], F32, tag="sc", bufs=4)
                            nc.tensor.matmul(
                                sc_a[:],
                                kT[:, skc * P : (skc + 1) * P],
                                qview[:, j, 0, 4 * qd : 4 * qd + 4, :],
                                start=True,
                                stop=True,
                            )
                            nc.scalar.activation(
                                at_t[:, 0:512],
                                sc_a[:],
                                mybir.ActivationFunctionType.Exp,
                                scale=0.125,
                            )
                        sc_b = psump.tile([P, 512], F32, tag="sc", bufs=4)
                        nc.tensor.matmul(
                            sc_b[:],
                            kT[:, skc * P : (skc + 1) * P],
                            qview[:, j, 1, 4 * qd : 4 * qd + 4, :],
                            start=True,
                            stop=True,
                        )
                        nc.scalar.activation(
                            at_t[:, (ncols - 512) : ncols],
                            sc_b[:],
                            mybir.ActivationFunctionType.Exp,
                            scale=0.125,
                        )
                        if skc == j:
                            for h4 in range(4):
                                nc.vector.tensor_mul(
                                    at_t[:, h4 * P : (h4 + 1) * P],
                                    at_t[:, h4 * P : (h4 + 1) * P],
                                    mkt[:, j * P : (j + 1) * P],
                                )
                        if skc == j + 4:
                            off = ncols - 512
                            for h4 in range(4):
                                nc.vector.tensor_mul(
                                    at_t[:, off + h4 * P : off + (h4 + 1) * P],
                                    at_t[:, off + h4 * P : off + (h4 + 1) * P],
                                    mkt[:, (j + 4) * P : (j + 5) * P],
                                )
                        ats.append((at_t, ncols))
                    # deferred normalization of the previous quad overlaps here
                    if pending is not None:
                        _normalize(j, *pending)
                        pending = None
                    # phase 2: dense attnv accumulation runs
                    oa_a = psump.tile([HD + 1, 512], F32, tag="oa", bufs=2)
                    oa_b = psump.tile([HD + 1, 512], F32, tag="oa", bufs=2)
                    for skc in range(j + 1):
                        at_t, ncols = ats[skc]
                        nc.tensor.matmul(
                            oa_a[:],
                            vaug[skc][:],
                            at_t[:, 0:512],
                            start=(skc == 0),
                            stop=(skc == j),
                            skip_group_check=True,
                        )
                    for skc in range(j + 5):
                        at_t, ncols = ats[skc]
                        nc.tensor.matmul(
                            oa_b[:],
                            vaug[skc][:],
                            at_t[:, (ncols - 512) : ncols],
                            start=(skc == 0),
                            stop=(skc == j + 4),
                            skip_group_check=True,
                        )
                    pending = (oa_a, oa_b, qd)
                if pending is not None:
                    _normalize(j, *pending)
                # ship this pair's oT (channel shard) and AllGather channels
                for c in range(4):
                    nc.sync.dma_start(
                        out=agin[j][c * P : (c + 1) * P, :].rearrange(
                            "p (a b) -> p a b", a=2
                        ),
                        in_=oT[c][:].rearrange("p (a b) -> p a b", b=P)[
                            :, j : j + 5 : 4, :
                        ],
                    )
                nc.gpsimd.collective_compute(
                    "AllGather",
                    mybir.AluOpType.bypass,
                    replica_groups=GROUPS,
                    ins=[agin[j].opt()],
                    outs=[agout[j].opt()],
                )
                pending_oproj.append(j)
                if len(pending_oproj) >= 2:
                    emit_oproj(pending_oproj.pop(0))
            while pending_oproj:
                emit_oproj(pending_oproj.pop(0))

    nc.compile()
    return nc


def _get_nc():
    if "nc" not in _NC_CACHE:
        _NC_CACHE["nc"] = _build_graph()
    return _NC_CACHE["nc"]


def _shard_inputs(x, wq, wk, wv, wo, cos, sin, mask, pos):
    x = np.asarray(x, dtype=np.float32)
    wq = np.asarray(wq, dtype=np.float32)
    wk = np.asarray(wk, dtype=np.float32)
    wv = np.asarray(wv, dtype=np.float32)
    wo = np.asarray(wo, dtype=np.float32)
    cos = np.asarray(cos, dtype=np.float32)
    sin = np.asarray(sin, dtype=np.float32)
    mask = np.asarray(mask, dtype=np.float32)
    p = int(pos)

    def pblock(a, nchunks):
        # [(chunks*128), n] -> [128, chunks, n] -> [128, chunks*n]
        n = a.shape[1]
        return np.ascontiguousarray(
            a.reshape(nchunks, P, n).transpose(1, 0, 2).reshape(P, nchunks * n)
        )

    cs = cos[p : p + S]  # [S, 32]
    sn = sin[p : p + S]
    cs8 = pblock(np.tile(cs, (1, NH)), SC)  # [128, 8*256]
    sn8 = pblock(np.tile(sn, (1, NH)), SC)
    # transposed diagonal 128x128 blocks of the additive mask, pre-scaled by
    # sqrt(HD) so exp(scale*(scores + 8*mask)) == exp(scores/8 + mask)
    mkb = np.concatenate(
        [
            (mask[j * P : (j + 1) * P, j * P : (j + 1) * P].T >= -0.5).astype(
                np.float32
            )
            for j in range(SC)
        ],
        axis=1,
    )
    mkb = np.ascontiguousarray(mkb)  # [128, 8*128]

    in_maps = []
    for d in range(N_CORES):
        g, kv = d // 4, d % 4
        in_maps.append(
            {
                "xt": np.ascontiguousarray(
                    x[g].T.reshape(DC, P, SC, P).transpose(2, 1, 0, 3).reshape(SC, P, D)
                ),
                "wq": pblock(wq[:, kv * 512 : (kv + 1) * 512], DC),
                "wkv": pblock(
                    np.concatenate(
                        [
                            wk[:, kv * HD : (kv + 1) * HD],
                            wv[:, kv * HD : (kv + 1) * HD],
                        ],
                        axis=1,
                    ),
                    DC,
                ),
                "wo": pblock(wo[:, kv * 512 : (kv + 1) * 512], DC),
                "cs8": cs8,
                "sn8": sn8,
                "mkb": mkb,
            }
        )
    return in_maps


def _run(inputs, trace=False, trace_kwargs=None):
    nc = _get_nc()
    in_maps = _shard_inputs(**inputs)
    res = run_bass_kernel_spmd(
        nc,
        in_maps,
        core_ids=list(range(N_CORES)),
        trace=trace,
        **(trace_kwargs or {}),
    )
    B = 2
    out = np.empty((B, S, D), dtype=np.float32)
    for d in range(N_CORES):
        g, kv = d // 4, d % 4
        core_out = res.results[d]["out"]  # [1024, 512]; rows 256j.. = pair j
        cols = slice(kv * 512, (kv + 1) * 512)
        for j in range(4):
            out[g, j * P : (j + 1) * P, cols] = core_out[256 * j : 256 * j + P]
            out[g, (j + 4) * P : (j + 5) * P, cols] = core_out[
                256 * j + P : 256 * j + 256
            ]
    return out, res


def kernel(**inputs) -> np.ndarray:
    out, _ = _run(inputs, trace=False)
    return out
